# revision 1
# baseline (speedup 1.0000x reference)
"""Decoder layer on 8 trn2 cores — fp8 DoubleRow edition.

Sharding: data-parallel over (batch b, sequence half h) -> core c = 2*b + h.
Each core computes the full decoder layer for its 1024 tokens; K/V are
recomputed for the visible prefix (zero collectives). The per-core KV buffer
is [own 1024 tokens | first-half 1024 tokens]; the prefix block is gated
on/off per core by an exp-bias input (0 or -1e30). Causal masking inside the
own block is via PSUM-preloaded -inf triangles (identity matmul, f32r).

QKV projections, QK^T scores, and AV run in fp8(e4m3) DoubleRow perf
mode (two 128-row k-subtiles per instruction); Q/K weights are scaled
x32 before quantization to dodge fp8 subnormals (compensated in the exp
scale / softmax-normalize constant). Scores use subtile 0 only (subtile
1 memset to zero once). AV folds the softmax denominator in via a ones
column interleaved per head in the V layout. The attention out-proj and
both MLP matmuls run in bf16 (fp8 there pushed absmax err too close to
the gate); wfc streams per column block, wpr is SBUF-resident. Weights
are pre-quantized host-side; activations quantize at PSUM-eviction time.
Everything runs transposed (xT [D, tokens]); LN stats via ones-column
matmuls; biases ride per-partition on eviction ops; residual adds are
DVE tensor_tensor on eviction, in place over the xo tiles (x1 == xo).
All intermediates stay SBUF-resident (no DRAM round-trips). Output
[D, tok]; host transposes.
"""

import numpy as np

D = 1024
H = 16
DH = 64
TQ = 1024
TKV = 2048
DFF = 4096
EPS = 1e-5
NEG = -1.0e30
KT = D // 128   # 8   128-row tiles of D
KP = KT // 2    # 4   256-row pairs of D
FP = 16         # 256-row pairs of DFF

_CACHE = {}


def _build():
    if "nc" in _CACHE:
        return _CACHE["nc"]
    import contextlib
    import concourse.mybir as mybir
    import concourse.tile as tile
    from concourse import bacc

    f32 = mybir.dt.float32
    f32r = mybir.dt.float32r
    fp8 = mybir.dt.float8e4
    bf16 = mybir.dt.bfloat16
    Act = mybir.ActivationFunctionType
    Alu = mybir.AluOpType
    DR = mybir.MatmulPerfMode.DoubleRow

    nc = bacc.Bacc(None, target_bir_lowering=False)

    # packed inputs: cstr = [masks(4x512) | ident(128) | onescol(1) | onesrow(row0,128) | bvr(row0,1024)]
    #                cstf = [pbias(1) | bqc(8) | bkc(8) | boc(8) | bprc(8) | bfcc(32)]
    #                wqkv8 = [wq(0:4) | wk(4:8) | wv(8:12)]; wb16 = [wo(0:8) | wfc(8:40) | wpr(40:72)]
    xkv = nc.declare_dram_parameter("xkv", [D, TKV], f32r, isOutput=False)
    cstr = nc.declare_dram_parameter("cstr", [128, 3329], f32r, isOutput=False)
    cstf = nc.declare_dram_parameter("cstf", [128, 65], f32, isOutput=False)
    wqkv8 = nc.declare_dram_parameter("wqkv8", [3 * KP, 128, 2, D], fp8, isOutput=False)
    wb16 = nc.declare_dram_parameter("wb16", [72, 128, D], bf16, isOutput=False)
    out_T = nc.declare_dram_parameter("out_T", [D, TQ], f32, isOutput=True)
    wq8 = [wqkv8[j] for j in range(KP)]
    wk8 = [wqkv8[KP + j] for j in range(KP)]
    wv8 = [wqkv8[2 * KP + j] for j in range(KP)]
    wo16 = [wb16[j] for j in range(KT)]
    wfc16 = [wb16[KT + j] for j in range(32)]
    wpr16 = [wb16[40 + j] for j in range(32)]

    with tile.TileContext(nc) as tc:
        with tc.tile_pool(name="const", bufs=1) as cst:
            id_t = cst.tile([128, 128], f32r)
            nc.sync.dma_start(out=id_t[:], in_=cstr[:, 2048:2176])
            ones_c = cst.tile([128, 1], f32r)
            nc.sync.dma_start(out=ones_c[:], in_=cstr[:, 2176:2177])
            ones_r = cst.tile([1, 128], f32r)
            nc.sync.dma_start(out=ones_r[:], in_=cstr[0:1, 2177:2305])
            pb_t = cst.tile([128, 1], f32)
            nc.sync.dma_start(out=pb_t[:], in_=cstf[:, 0:1])
            mask_t = []
            for j in range(4):
                m = cst.tile([128, 512], f32r, tag=f"mask{j}")
                nc.sync.dma_start(out=m[:], in_=cstr[:, j * 512:(j + 1) * 512])
                mask_t.append(m)

            # persistent activation storage (fp8); lifetimes managed explicitly
            xo_stack = contextlib.ExitStack()
            xop = xo_stack.enter_context(tc.tile_pool(name="xo", bufs=1))
            xo = [xop.tile([128, TQ], f32r, tag=f"xo{i}", name=f"xo{i}") for i in range(KT)]
            qkv_stack = contextlib.ExitStack()
            qdp = qkv_stack.enter_context(tc.tile_pool(name="qd", bufs=1))
            kdp = qkv_stack.enter_context(tc.tile_pool(name="kd", bufs=1))
            vvp = qkv_stack.enter_context(tc.tile_pool(name="vp", bufs=1))
            q_dup = [qdp.tile([128, 2, TQ], fp8, tag=f"qd{i}", name=f"qd{i}") for i in range(KT)]
            k_dup = [kdp.tile([128, 2, TKV], fp8, tag=f"kd{i}", name=f"kd{i}") for i in range(KT)]
            v_pair = [vvp.tile([128, 2, 16, 65], fp8, tag=f"vp{i}", name=f"vp{i}") for i in range(KT)]
            for p in range(KT):
                nc.any.memset(v_pair[p][:, :, :, 64:65], 32.0)
                nc.any.memset(q_dup[p][:, 1, :], 0.0)
                nc.any.memset(k_dup[p][:, 1, :], 0.0)

            def ln_pools(s):
                return dict(
                    sq=s.enter_context(tc.tile_pool(name="sq", bufs=3)),
                    st=s.enter_context(tc.tile_pool(name="st", bufs=3)),
                    bc=s.enter_context(tc.tile_pool(name="bc", bufs=1)),
                    pst=s.enter_context(tc.tile_pool(name="pst", bufs=4, space="PSUM")),
                )

            def layernorm(P, src_fn, out_fn):
                """src_fn(k) -> tile [128, TQ] f32r; out_fn(k) -> dest AP [128, TQ]."""
                nch = TQ // 512
                ps1 = [P["pst"].tile([1, 512], f32, tag="st", name=f"ps1_{_i}") for _i in range(nch)]
                ps2 = [P["pst"].tile([1, 512], f32, tag="st", name=f"ps2_{_i}") for _i in range(nch)]
                for k in range(KT):
                    src = src_fn(k)
                    sqt = P["sq"].tile([128, TQ], f32r, tag="sqs")
                    nc.scalar.activation(sqt[:], src[:], Act.Square)
                    for c in range(nch):
                        nc.tensor.matmul(ps1[c][:], ones_c[:], src[:, c * 512:(c + 1) * 512],
                                         start=(k == 0), stop=(k == KT - 1))
                        nc.tensor.matmul(ps2[c][:], ones_c[:], sqt[:, c * 512:(c + 1) * 512],
                                         start=(k == 0), stop=(k == KT - 1))
                mu = P["st"].tile([1, TQ], f32r, tag="mu", bufs=1)
                ex2 = P["st"].tile([1, TQ], f32, tag="chain")
                for c in range(nch):
                    nc.scalar.mul(mu[:, c * 512:(c + 1) * 512], ps1[c][:], 1.0 / D)
                    nc.scalar.mul(ex2[:, c * 512:(c + 1) * 512], ps2[c][:], 1.0 / D)
                mu2 = P["st"].tile([1, TQ], f32, tag="chain")
                nc.vector.tensor_tensor(mu2[:], mu[:].bitcast(f32), mu[:].bitcast(f32), Alu.mult)
                var = P["st"].tile([1, TQ], f32, tag="chain")
                nc.vector.tensor_tensor(var[:], ex2[:], mu2[:], Alu.subtract)
                vre = P["st"].tile([1, TQ], f32, tag="chain")
                nc.vector.tensor_scalar_add(vre[:], var[:], EPS)
                vri = P["st"].tile([1, TQ], f32, tag="chain")
                nc.vector.reciprocal(vri[:], vre[:])
                rs = P["st"].tile([1, TQ], f32r, tag="rs", bufs=1)
                nc.scalar.activation(rs[:], vri[:], Act.Sqrt)
                mu_bc = P["bc"].tile([128, TQ], f32r, tag="mubc")
                rs_bc = P["bc"].tile([128, TQ], f32r, tag="rsbc")
                nc.gpsimd.partition_broadcast(mu_bc[:], mu[:])
                nc.gpsimd.partition_broadcast(rs_bc[:], rs[:])
                for k in range(KT):
                    src = src_fn(k)
                    t1 = P["sq"].tile([128, TQ], f32, tag="sqs")
                    nc.gpsimd.tensor_sub(t1[:], src[:].bitcast(f32), mu_bc[:].bitcast(f32))
                    nc.vector.tensor_tensor(out_fn(k), t1[:], rs_bc[:].bitcast(f32), Alu.mult)

            # ============ Scope 1: LN1 + QKV projections (per token half)
            with contextlib.ExitStack() as s1:
                P = ln_pools(s1)
                h1p = s1.enter_context(tc.tile_pool(name="h1", bufs=2))
                xpp = s1.enter_context(tc.tile_pool(name="xp", bufs=4))
                wqp = s1.enter_context(tc.tile_pool(name="wqkv", bufs=1))
                bqp = s1.enter_context(tc.tile_pool(name="bq", bufs=1))
                pmm = s1.enter_context(tc.tile_pool(name="pmm", bufs=4, space="PSUM"))
                bq_sb = bqp.tile([128, KT], f32, tag="bqs")
                nc.sync.dma_start(out=bq_sb[:], in_=cstf[:, 1:9])
                bk_sb = bqp.tile([128, KT], f32, tag="bks")
                nc.sync.dma_start(out=bk_sb[:], in_=cstf[:, 9:17])
                bv_t = bqp.tile([1, D], f32r, tag="bvt")
                nc.sync.dma_start(out=bv_t[:], in_=cstr[0:1, 2305:3329])
                wq8t, wk8t, wv8t = [], [], []
                for j in range(KP):
                    for nm, src, lst in (("wq", wq8, wq8t), ("wk", wk8, wk8t), ("wv", wv8, wv8t)):
                        wt = wqp.tile([128, 2, D], fp8, tag=f"{nm}{j}")
                        nc.sync.dma_start(out=wt[:], in_=src[j])
                        lst.append(wt)
                for half in range(2):
                    if half == 0:
                        for k in range(KT):
                            nc.sync.dma_start(out=xo[k][:], in_=xkv[k * 128:(k + 1) * 128, 0:TQ])
                        src_fn = lambda k: xo[k]
                    else:
                        def src_fn(k):
                            t = xpp.tile([128, TQ], f32r, tag="xps")
                            nc.sync.dma_start(out=t[:], in_=xkv[k * 128:(k + 1) * 128, TQ:TKV])
                            return t
                    h18 = [h1p.tile([128, 2, TQ], fp8, tag=f"h1{j}", name=f"h1t{half}_{j}") for j in range(KP)]
                    layernorm(P, src_fn, lambda k: h18[k // 2][:, k % 2, :])
                    if half == 0:
                        for mc in range(KT):
                            for c in range(2):
                                ps = pmm.tile([128, 512], f32, tag="pmm")
                                for j in range(KP):
                                    nc.tensor.matmul(ps[:], wq8t[j][:, :, mc * 128:(mc + 1) * 128],
                                                     h18[j][:, :, c * 512:(c + 1) * 512],
                                                     start=(j == 0), stop=(j == KP - 1), perf_mode=DR)
                                nc.vector.tensor_scalar_add(q_dup[mc][:, 0, c * 512:(c + 1) * 512],
                                                             ps[:], bq_sb[:, mc:mc + 1])
                    for mc in range(KT):
                        for cg in range(2):
                            cglob = half * 2 + cg
                            ps = pmm.tile([128, 512], f32, tag="pmm")
                            for j in range(KP):
                                nc.tensor.matmul(ps[:], wk8t[j][:, :, mc * 128:(mc + 1) * 128],
                                                 h18[j][:, :, cg * 512:(cg + 1) * 512],
                                                 start=(j == 0), stop=(j == KP - 1), perf_mode=DR)
                            nc.vector.tensor_scalar_add(k_dup[mc][:, 0, cglob * 512:(cglob + 1) * 512],
                                                         ps[:], bk_sb[:, mc:mc + 1])
                    for tl in range(8):
                        tt = half * 8 + tl
                        for c in range(2):
                            ps = pmm.tile([128, 512], f32, tag="pmm")
                            nc.tensor.matmul(ps[:], ones_r[:], bv_t[:, c * 512:(c + 1) * 512],
                                             start=True, stop=False)
                            for j in range(KP):
                                nc.tensor.matmul(ps[:], h18[j][:, :, tl * 128:(tl + 1) * 128],
                                                 wv8t[j][:, :, c * 512:(c + 1) * 512],
                                                 start=False, stop=(j == KP - 1), perf_mode=DR)
                            nc.scalar.copy(v_pair[tt // 2][:, tt % 2, c * 8:(c + 1) * 8, 0:64], ps[:])

            # ============ Scope 2: attention
            PAIRS = {0: [0, 1, 4, 5, 6, 7], 1: [0, 1, 2, 3, 4, 5, 6, 7]}
            op_stack = contextlib.ExitStack()
            wop = op_stack.enter_context(tc.tile_pool(name="wo", bufs=1))
            t1p = op_stack.enter_context(tc.tile_pool(name="t1", bufs=1))
            bop = op_stack.enter_context(tc.tile_pool(name="bo", bufs=1))
            pm2 = op_stack.enter_context(tc.tile_pool(name="pm2", bufs=1, space="PSUM"))
            bo_sb = bop.tile([128, KT], f32, tag="bos")
            nc.sync.dma_start(out=bo_sb[:], in_=cstf[:, 17:25])
            wo8t = []
            for j in range(KT):
                wt = wop.tile([128, D], bf16, tag=f"wo{j}", name=f"wo{j}")
                nc.sync.dma_start(out=wt[:], in_=wo16[j])
                wo8t.append(wt)
            t1t = [t1p.tile([128, TQ], f32, tag=f"t1{i}", name=f"t1{i}") for i in range(KT)]
            with nc.allow_low_precision(reason="x1 residual kept in f32r for LN2 stats matmuls"):
                for mc in range(KT):
                    nc.vector.tensor_scalar_add(xo[mc][:], xo[mc][:].bitcast(f32),
                                                bo_sb[:, mc:mc + 1])
            at_stack = contextlib.ExitStack()
            atp = at_stack.enter_context(tc.tile_pool(name="at8", bufs=1))
            with contextlib.ExitStack() as s2:
                etp = s2.enter_context(tc.tile_pool(name="et", bufs=6))
                stp = s2.enter_context(tc.tile_pool(name="st2", bufs=2))
                pss = s2.enter_context(tc.tile_pool(name="pss", bufs=2, space="PSUM"))
                pav = s2.enter_context(tc.tile_pool(name="pav", bufs=2, space="PSUM"))
                attnb = [atp.tile([128, TQ], bf16, tag=f"at{i}", name=f"at{i}") for i in range(KT)]
                def outproj_partial(jlo, jhi, emit):
                    for mc in range(KT):
                        ps = pm2.tile([128, TQ], f32, tag="pm2")
                        for j in range(jlo, jhi):
                            for c in range(2):
                                nc.tensor.matmul(ps[:, c * 512:(c + 1) * 512],
                                                 wo8t[j][:, mc * 128:(mc + 1) * 128],
                                                 attnb[j][:, c * 512:(c + 1) * 512],
                                                 start=(j == jlo), stop=(j == jhi - 1))
                        emit(mc, ps)
                for h in range(H):
                    mcK = h // 2
                    off = (h % 2) * 64
                    for qc in range(2):
                        ets = {}
                        for p in PAIRS[qc]:
                            ps2 = pss.tile([128, 1024], f32, tag="pss")
                            for i in range(2):
                                kt = 2 * p + i
                                bnd = 4 * qc <= kt < 4 * (qc + 1)
                                dst = ps2[:, i * 512:(i + 1) * 512]
                                if bnd:
                                    nc.tensor.matmul(dst, id_t[:], mask_t[kt - 4 * qc][:], start=True, stop=False)
                                nc.tensor.matmul(dst, k_dup[mcK][off:off + 64, :, kt * 128:(kt + 1) * 128],
                                                 q_dup[mcK][off:off + 64, :, qc * 512:(qc + 1) * 512],
                                                 start=(not bnd), stop=True, perf_mode=DR)
                            et = etp.tile([128, 2, 512], fp8, tag="et")
                            bias = pb_t[:, 0:1] if p >= 4 else 0.0
                            nc.scalar.activation(et[:], ps2[:], Act.Exp, bias=bias, scale=0.125 / 1024.0)
                            ets[p] = et
                        ps_av = pav.tile([65, 512], f32, tag="pav")
                        vis = PAIRS[qc]
                        for idx, p in enumerate(vis):
                            nc.tensor.matmul(ps_av[:], v_pair[p][:, :, h, 0:65], ets[p][:],
                                             start=(idx == 0), stop=(idx == len(vis) - 1), perf_mode=DR)
                        rec = stp.tile([1, 512], f32, tag="rec")
                        nc.vector.reciprocal(rec[:], ps_av[64:65, :])
                        bc_sb = stp.tile([64, 512], f32, tag="bcsb")
                        nc.gpsimd.partition_broadcast(bc_sb[:], rec[:], channels=64)
                        nc.vector.tensor_tensor(attnb[mcK][off:off + 64, qc * 512:(qc + 1) * 512],
                                                ps_av[0:64, :], bc_sb[:], Alu.mult)
                    if h in (3, 7, 11):
                        # partial out-proj over completed head-pairs overlaps later exp
                        jlo, jhi = (h - 3) // 2, (h + 1) // 2
                        if h == 3:
                            outproj_partial(jlo, jhi, lambda mc, ps: nc.vector.tensor_tensor(
                                t1t[mc][:], ps[:], xo[mc][:].bitcast(f32), Alu.add))
                        else:
                            outproj_partial(jlo, jhi, lambda mc, ps: nc.vector.tensor_tensor(
                                t1t[mc][:], ps[:], t1t[mc][:], Alu.add))

            # ============ Scope 2b: wave-2 out-proj partial + combine -> x1
            # (x1 reuses the xo tiles in place: x1 = x + attn_out + bo)
            x1 = xo
            def emit_combine(mc, ps):
                with nc.allow_low_precision(reason="x1 residual kept in f32r for LN2 stats matmuls"):
                    nc.vector.tensor_tensor(x1[mc][:], ps[:], t1t[mc][:], Alu.add)
            outproj_partial(KT - 2, KT, emit_combine)
            at_stack.close()
            op_stack.close()
            qkv_stack.close()

            # ============ Scope 3a: LN2 -> h2 (bf16)
            h2_stack = contextlib.ExitStack()
            h2p = h2_stack.enter_context(tc.tile_pool(name="h2", bufs=1))
            h2b = [h2p.tile([128, TQ], bf16, tag=f"h2{j}", name=f"h2t{j}") for j in range(KT)]
            with contextlib.ExitStack() as s3:
                P = ln_pools(s3)
                layernorm(P, lambda k: x1[k], lambda k: h2b[k][:])

            # ============ Scope 3b: MLP (bf16)
            with contextlib.ExitStack() as s3b:
                wfp = s3b.enter_context(tc.tile_pool(name="wf", bufs=4))
                mtp = s3b.enter_context(tc.tile_pool(name="mt", bufs=1))
                bfp = s3b.enter_context(tc.tile_pool(name="bf", bufs=1))
                evp = s3b.enter_context(tc.tile_pool(name="ev", bufs=3))
                pm3 = s3b.enter_context(tc.tile_pool(name="pm3", bufs=4, space="PSUM"))
                bfc_sb = bfp.tile([128, 32], f32, tag="bfs")
                nc.sync.dma_start(out=bfc_sb[:], in_=cstf[:, 33:65])
                bpr_sb = bfp.tile([128, KT], f32, tag="bps")
                nc.sync.dma_start(out=bpr_sb[:], in_=cstf[:, 25:33])
                wpp2 = s3b.enter_context(tc.tile_pool(name="wpB", bufs=1))
                wprt = []
                for j in range(32):
                    wt = wpp2.tile([128, D], bf16, tag=f"wp{j}", name=f"wpt{j}")
                    nc.sync.dma_start(out=wt[:], in_=wpr16[j])
                    wprt.append(wt)
                with nc.allow_low_precision(reason="x1+bias kept in f32r"):
                    for mc in range(KT):
                        nc.vector.tensor_scalar_add(x1[mc][:], x1[mc][:].bitcast(f32),
                                                    bpr_sb[:, mc:mc + 1])
                mtb = [mtp.tile([128, TQ], bf16, tag=f"mt{j}", name=f"mtt{j}") for j in range(32)]
                for hc in range(32):
                    wf_t = wfp.tile([128, KT * 128], bf16, tag="wft")
                    nc.sync.dma_start(out=wf_t[:], in_=wfc16[hc])
                    ps = pm3.tile([128, TQ], f32, tag="pm3")
                    for j in range(KT):
                        for c in range(2):
                            nc.tensor.matmul(ps[:, c * 512:(c + 1) * 512],
                                             wf_t[:, j * 128:(j + 1) * 128],
                                             h2b[j][:, c * 512:(c + 1) * 512],
                                             start=(j == 0), stop=(j == KT - 1))
                    nc.scalar.activation(mtb[hc][:], ps[:], Act.Gelu, bias=bfc_sb[:, hc:hc + 1])
                for mc in range(KT):
                    ps = pm3.tile([128, TQ], f32, tag="pm3")
                    for j in range(32):
                        for c in range(2):
                            nc.tensor.matmul(ps[:, c * 512:(c + 1) * 512],
                                             wprt[j][:, mc * 128:(mc + 1) * 128],
                                             mtb[j][:, c * 512:(c + 1) * 512],
                                             start=(j == 0), stop=(j == 31))
                    o = evp.tile([128, TQ], f32, tag="o")
                    nc.vector.tensor_tensor(o[:], ps[:], x1[mc][:].bitcast(f32), Alu.add)
                    nc.sync.dma_start(out=out_T[mc * 128:(mc + 1) * 128, :], in_=o[:])
            h2_stack.close()
            xo_stack.close()

    nc.compile()
    _CACHE["nc"] = nc
    return nc


def make_in_maps(x, ln1_g, ln1_b, wq, wk, wv, wo, bo, ln2_g, ln2_b, w_fc, b_fc, w_pr, b_pr):
    import ml_dtypes
    f8 = np.dtype(ml_dtypes.float8_e4m3)
    b16 = np.dtype(ml_dtypes.bfloat16)
    x = np.asarray(x, np.float32)

    def q8(a, npair):
        a = np.clip(np.asarray(a, np.float32), -240.0, 240.0).astype(f8)
        R, C = a.shape
        assert R == npair * 256
        return np.ascontiguousarray(a.reshape(npair, 2, 128, C).transpose(0, 2, 1, 3))

    def cols(b, n):
        return np.ascontiguousarray(np.asarray(b, np.float32).reshape(n, 128).T)

    mk = np.zeros((4, 128, 512), np.float32)
    for j in range(4):
        kp = np.arange(128)[:, None] + j * 128
        qf = np.arange(512)[None, :]
        mk[j] = np.where(kp <= qf, 0.0, NEG)

    g1v = np.asarray(ln1_g, np.float32)
    b1v = np.asarray(ln1_b, np.float32)
    g2v = np.asarray(ln2_g, np.float32)
    b2v = np.asarray(ln2_b, np.float32)
    wq2 = np.transpose(np.asarray(wq, np.float32), (1, 0, 2)).reshape(D, D)
    wk2 = np.transpose(np.asarray(wk, np.float32), (1, 0, 2)).reshape(D, D)
    wv2 = np.transpose(np.asarray(wv, np.float32), (1, 0, 2)).reshape(D, D)
    wfc2 = np.asarray(w_fc, np.float32)

    cstr = np.zeros((128, 3329), np.float32)
    cstr[:, 0:2048] = mk.transpose(1, 0, 2).reshape(128, 2048)
    cstr[:, 2048:2176] = np.eye(128, dtype=np.float32)
    cstr[:, 2176:2177] = 1.0
    cstr[0, 2177:2305] = 1.0
    cstr[0, 2305:3329] = 32.0 * (b1v @ wv2)
    cstf = np.zeros((128, 65), np.float32)
    cstf[:, 1:9] = cols(32.0 * (b1v @ wq2), KT)
    cstf[:, 9:17] = cols(32.0 * (b1v @ wk2), KT)
    cstf[:, 17:25] = cols(np.asarray(bo, np.float32), KT)
    cstf[:, 25:33] = cols(np.asarray(b_pr, np.float32), KT)
    cstf[:, 33:65] = cols(np.asarray(b_fc, np.float32) + b2v @ wfc2, 32)
    wqkv8 = np.concatenate([
        q8(32.0 * g1v[:, None] * wq2, KP),
        q8(32.0 * g1v[:, None] * wk2, KP),
        q8(32.0 * g1v[:, None] * wv2, KP),
    ], 0)
    wb16 = np.concatenate([
        np.asarray(wo, np.float32).reshape(KT, 128, D).astype(b16),
        np.ascontiguousarray(
            (g2v[:, None] * wfc2).reshape(KT, 128, 32, 128).transpose(2, 1, 0, 3)
        ).reshape(32, 128, KT * 128).astype(b16),
        np.asarray(w_pr, np.float32).reshape(32, 128, D).astype(b16),
    ], 0)
    shared = {"cstr": cstr, "wqkv8": wqkv8, "wb16": wb16}
    in_maps = []
    for b in range(4):
        for h in range(2):
            own = x[b, h * TQ:(h + 1) * TQ]
            pref = x[b, 0:TQ]
            m = dict(shared)
            m["xkv"] = np.ascontiguousarray(np.concatenate([own, pref], 0).T)
            cf = cstf.copy()
            cf[:, 0] = 0.0 if h == 1 else NEG
            m["cstf"] = cf
            in_maps.append(m)
    return in_maps


def kernel(x, ln1_g, ln1_b, wq, wk, wv, wo, bo, ln2_g, ln2_b, w_fc, b_fc, w_pr, b_pr):
    from concourse.bass_utils import run_bass_kernel_spmd

    nc = _build()
    in_maps = make_in_maps(x, ln1_g, ln1_b, wq, wk, wv, wo, bo, ln2_g, ln2_b,
                           w_fc, b_fc, w_pr, b_pr)
    res = run_bass_kernel_spmd(nc, in_maps, list(range(8)))
    out = np.empty((4, 2048, D), np.float32)
    for b in range(4):
        for h in range(2):
            out[b, h * TQ:(h + 1) * TQ, :] = res.results[2 * b + h]["out_T"].T
    return out



# revision 28
# speedup vs baseline: 4.0904x; 4.0904x over previous
"""Decoder layer on 8 trn2 cores — fp8 DoubleRow edition, v2.

Sharding: data-parallel; core c = 2*b + a handles batch b and query-block
set a: a=0 -> 512-token blocks {0, 3}, a=1 -> blocks {1, 2} (balanced causal
work: each core sees 12 visible 256-kv x 512-q pair-blocks per head vs 14
for the contiguous split). K/V are recomputed for the full 2048 tokens of
the batch (own blocks first, then the other core's blocks); per-core
visibility of the "other" kv pairs is gated by two exp-bias inputs
(pb0 gates pairs 4,5 in qc0; pb1 gates pairs 6,7 in qc1). Causal masking
inside the diagonal blocks is via PSUM-preloaded -inf triangles (identity
matmul, f32r).

QKV projections and AV run in fp8(e4m3) DoubleRow (true 256-row contraction
per instruction); QK^T scores run fp8 WITHOUT DoubleRow (contraction is only
DH=64, so DR bought nothing and its LDWEIGHTS cost 8x FWL's). Q/K weights are
scaled x32 before quantization to dodge fp8 subnormals (compensated in the
exp scale / softmax-normalize constant). AV folds the softmax denominator in
via a ones column interleaved per head in the V layout.

All biases (bq, bk, bv, bo, bpr) are seeded into PSUM via [1,128]x[1,512]
rank-1 matmuls at accumulation start - no elementwise bias passes, no
in-place residual mutation. LN stats via ones-column matmuls; the normalize
multiplies use PE-generated broadcast tiles (ones x row matmul -> PSUM ->
SBUF copy) instead of gpsimd partition_broadcast, with DVE-only per-tile
ops (x*rs in f32 -> bf16 tmp, tmp - mu*rs in bf16 2x mode). rsqrt via ACT
Rsqrt. The attention out-proj runs in 5 partial waves (after heads 3, 7,
11, 13, and at the end) overlapping the exp stream. MLP in bf16, wfc
streamed per column block, wpr SBUF-resident. Everything runs transposed
(xT [D, tokens]); intermediates stay SBUF-resident. Output [D, tok]; host
transposes and reassembles blocks.
"""

import numpy as np

D = 1024
H = 16
DH = 64
TQ = 1024
TKV = 2048
DFF = 4096
EPS = 1e-5
NEG = -1.0e30
KT = D // 128   # 8   128-row tiles of D
KP = KT // 2    # 4   256-row pairs of D
NCH = TQ // 512  # 2

# rows param layout (row vectors on partition 0): [ones(512) | bv | bq | bk | bo | bpr]
R_ONES = 0
R_BV = 512
R_BQ = R_BV + D
R_BK = R_BQ + D
R_BO = R_BK + D
R_BPR = R_BO + D
R_W = R_BPR + D  # 5632

_CACHE = {}


def _build():
    if "nc" in _CACHE:
        return _CACHE["nc"]
    import contextlib
    import concourse.mybir as mybir
    import concourse.tile as tile
    from concourse import bacc

    f32 = mybir.dt.float32
    f32r = mybir.dt.float32r
    fp8 = mybir.dt.float8e4
    bf16 = mybir.dt.bfloat16
    Act = mybir.ActivationFunctionType
    Alu = mybir.AluOpType
    DR = mybir.MatmulPerfMode.DoubleRow

    nc = bacc.Bacc(None, target_bir_lowering=False)

    # packed inputs:
    #   cstr = [masks(4x512) | ident(128) | onescol(1)]              [128, 2177] f32r
    #   rows = [ones(512) | bv | bq | bk | bo | bpr]                 [1, 5632]  f32r
    #   cstf = [pb0(1) | pb1(1) | bfcc(32)]                          [128, 34]  f32
    #   wqkv8 = [wq(0:4) | wk(4:8) | wv(8:12)]; wb16 = [wo(0:8) | wfc(8:40) | wpr(40:72)]
    xkv = nc.declare_dram_parameter("xkv", [D, TKV], f32r, isOutput=False)
    cstr = nc.declare_dram_parameter("cstr", [128, 2177], f32r, isOutput=False)
    rows = nc.declare_dram_parameter("rows", [1, R_W], f32r, isOutput=False)
    cstf = nc.declare_dram_parameter("cstf", [128, 34], f32, isOutput=False)
    wqkv8 = nc.declare_dram_parameter("wqkv8", [3 * KP, 128, 2, D], fp8, isOutput=False)
    wb16 = nc.declare_dram_parameter("wb16", [72, 128, D], bf16, isOutput=False)
    out_T = nc.declare_dram_parameter("out_T", [D, TQ], f32, isOutput=True)
    wq8 = [wqkv8[j] for j in range(KP)]
    wk8 = [wqkv8[KP + j] for j in range(KP)]
    wv8 = [wqkv8[2 * KP + j] for j in range(KP)]
    wo16 = [wb16[j] for j in range(KT)]
    wfc16 = [wb16[KT + j] for j in range(32)]
    wpr16 = [wb16[40 + j] for j in range(32)]

    with tile.TileContext(nc) as tc:
        with tc.tile_pool(name="const", bufs=1) as cst:
            ones_c = cst.tile([128, 1], f32r)
            nc.sync.dma_start(out=ones_c[:], in_=cstr[:, 2176:2177])
            ones_t = cst.tile([1, 512], f32r, tag="onesr")
            nc.sync.dma_start(out=ones_t[:], in_=rows[0:1, R_ONES:R_ONES + 512])
            pb_t = cst.tile([128, 2], f32)
            nc.sync.dma_start(out=pb_t[:], in_=cstf[:, 0:2])
            ones_r = ones_t[0:1, 0:512]

            # persistent activation storage; lifetimes managed explicitly
            import contextlib as _ctx
            xo_stack = _ctx.ExitStack()
            xop = xo_stack.enter_context(tc.tile_pool(name="xo", bufs=1))
            xo = [xop.tile([128, TQ], f32r, tag=f"xo{i}", name=f"xo{i}") for i in range(KT)]
            qkv_stack = _ctx.ExitStack()
            qdp = qkv_stack.enter_context(tc.tile_pool(name="qd", bufs=1))
            kdp = qkv_stack.enter_context(tc.tile_pool(name="kd", bufs=1))
            vvp = qkv_stack.enter_context(tc.tile_pool(name="vp", bufs=1))
            q8t = [qdp.tile([128, TQ], fp8, tag=f"qd{i}", name=f"qd{i}") for i in range(KT)]
            k8t = [kdp.tile([128, TKV], fp8, tag=f"kd{i}", name=f"kd{i}") for i in range(KT)]
            v_pair = [vvp.tile([128, 2, 16, 65], fp8, tag=f"vp{i}", name=f"vp{i}") for i in range(KT)]
            for p in range(KT):
                nc.any.memset(v_pair[p][:, :, :, 64:65], 32.0)

            def ln_pools(s, pbc=None):
                return dict(
                    sq=s.enter_context(tc.tile_pool(name="sq", bufs=2)),
                    st=s.enter_context(tc.tile_pool(name="st", bufs=3)),
                    bcs=s.enter_context(tc.tile_pool(name="bc", bufs=1)),
                    pst=s.enter_context(tc.tile_pool(name="pst", bufs=4, space="PSUM")),
                    pbc=(pbc if pbc is not None else
                         s.enter_context(tc.tile_pool(name="pbc", bufs=2, space="PSUM"))),
                )

            def layernorm(P, src_fn, out_fn):
                """src_fn(k) -> tile [128, TQ] f32r; out_fn(k) -> dest AP [128, TQ]."""
                ps1 = [P["pst"].tile([1, 512], f32, tag="st", name=f"ps1_{_i}") for _i in range(NCH)]
                ps2 = [P["pst"].tile([1, 512], f32, tag="st", name=f"ps2_{_i}") for _i in range(NCH)]
                for k in range(KT):
                    src = src_fn(k)
                    sqt = P["sq"].tile([128, TQ], f32r, tag="sqs")
                    nc.scalar.activation(sqt[:], src[:], Act.Square)
                    for c in range(NCH):
                        nc.tensor.matmul(ps1[c][:], ones_c[:], src[:, c * 512:(c + 1) * 512],
                                         start=(k == 0), stop=(k == KT - 1))
                        nc.tensor.matmul(ps2[c][:], ones_c[:], sqt[:, c * 512:(c + 1) * 512],
                                         start=(k == 0), stop=(k == KT - 1))
                mu = P["st"].tile([1, TQ], f32r, tag="mu", bufs=1)
                ex2 = P["st"].tile([1, TQ], f32, tag="chain")
                for c in range(NCH):
                    nc.scalar.mul(mu[:, c * 512:(c + 1) * 512], ps1[c][:], 1.0 / D)
                    # ex2 + EPS in one op: Copy(ps2/D + EPS)
                    nc.scalar.activation(ex2[:, c * 512:(c + 1) * 512], ps2[c][:],
                                         Act.Copy, bias=EPS, scale=1.0 / D)
                mu2 = P["st"].tile([1, TQ], f32, tag="chain")
                nc.vector.tensor_tensor(mu2[:], mu[:].bitcast(f32), mu[:].bitcast(f32), Alu.mult)
                var = P["st"].tile([1, TQ], f32, tag="chain")
                nc.vector.tensor_tensor(var[:], ex2[:], mu2[:], Alu.subtract)
                vri = P["st"].tile([1, TQ], f32, tag="chain")
                nc.vector.reciprocal(vri[:], var[:])
                rs = P["st"].tile([1, TQ], f32r, tag="rs", bufs=1)
                nc.scalar.activation(rs[:], vri[:], Act.Sqrt)
                murs = P["st"].tile([1, TQ], f32r, tag="murs", bufs=1)
                with nc.allow_low_precision(reason="murs row kept f32r for broadcast matmul rhs"):
                    nc.vector.tensor_tensor(murs[:], mu[:].bitcast(f32), rs[:].bitcast(f32), Alu.mult)
                # broadcast via PE rank-1 matmuls -> PSUM -> SBUF copies
                rs_bc = P["bcs"].tile([128, TQ], f32, tag="rsbc")
                mrs_bc = P["bcs"].tile([128, TQ], bf16, tag="mrsbc")
                for c in range(NCH):
                    pb1t = P["pbc"].tile([128, 512], f32, tag="pmm")
                    nc.tensor.matmul(pb1t[:], ones_r[0:1, 0:128], rs[0:1, c * 512:(c + 1) * 512],
                                     start=True, stop=True)
                    nc.vector.tensor_copy(rs_bc[:, c * 512:(c + 1) * 512], pb1t[:])
                    pb2t = P["pbc"].tile([128, 512], f32, tag="pmm")
                    nc.tensor.matmul(pb2t[:], ones_r[0:1, 0:128], murs[0:1, c * 512:(c + 1) * 512],
                                     start=True, stop=True)
                    nc.vector.tensor_copy(mrs_bc[:, c * 512:(c + 1) * 512], pb2t[:])
                for k in range(KT):
                    src = src_fn(k)
                    t1 = P["sq"].tile([128, TQ], bf16, tag="tmp")
                    nc.vector.tensor_tensor(t1[:], src[:].bitcast(f32), rs_bc[:], Alu.mult)
                    nc.vector.tensor_tensor(out_fn(k), t1[:], mrs_bc[:], Alu.subtract)

            # ============ Scope 1: LN1 + QKV projections (per token half)
            with contextlib.ExitStack() as s1:
                h1p = s1.enter_context(tc.tile_pool(name="h1", bufs=2))
                xpp = s1.enter_context(tc.tile_pool(name="xp", bufs=4))
                wqp = s1.enter_context(tc.tile_pool(name="wqkv", bufs=1))
                pmm = s1.enter_context(tc.tile_pool(name="pmm", bufs=4, space="PSUM"))
                P = ln_pools(s1, pbc=pmm)
                rqkv = wqp.tile([1, 3 * D], f32r, tag="rqkv")
                nc.sync.dma_start(out=rqkv[:], in_=rows[0:1, R_BV:R_BV + 3 * D])
                wq8t, wk8t, wv8t = [], [], []
                for j in range(KP):
                    for nm, src, lst in (("wq", wq8, wq8t), ("wk", wk8, wk8t), ("wv", wv8, wv8t)):
                        wt = wqp.tile([128, 2, D], fp8, tag=f"{nm}{j}")
                        nc.sync.dma_start(out=wt[:], in_=src[j])
                        lst.append(wt)
                for k in range(KT):
                    nc.sync.dma_start(out=xo[k][:], in_=xkv[k * 128:(k + 1) * 128, 0:TQ])

                def xp_fn(k):
                    t = xpp.tile([128, TQ], f32r, tag="xps")
                    nc.sync.dma_start(out=t[:], in_=xkv[k * 128:(k + 1) * 128, TQ:TKV])
                    return t
                for half in range(2):
                    src_fn = (lambda k: xo[k]) if half == 0 else xp_fn
                    h18 = [h1p.tile([128, 2, TQ], fp8, tag=f"h1{j}", name=f"h1t{half}_{j}") for j in range(KP)]
                    layernorm(P, src_fn, lambda k: h18[k // 2][:, k % 2, :])
                    if half == 0:
                        for mc in range(KT):
                            for c in range(2):
                                ps = pmm.tile([128, 512], f32, tag="pmm")
                                nc.tensor.matmul(ps[:], rqkv[0:1, D + mc * 128:D + (mc + 1) * 128],
                                                 ones_r, start=True, stop=False)
                                for j in range(KP):
                                    nc.tensor.matmul(ps[:], wq8t[j][:, :, mc * 128:(mc + 1) * 128],
                                                     h18[j][:, :, c * 512:(c + 1) * 512],
                                                     start=False, stop=(j == KP - 1), perf_mode=DR)
                                nc.vector.tensor_copy(q8t[mc][:, c * 512:(c + 1) * 512], ps[:])
                    for mc in range(KT):
                        for cg in range(2):
                            cglob = half * 2 + cg
                            ps = pmm.tile([128, 512], f32, tag="pmm")
                            nc.tensor.matmul(ps[:], rqkv[0:1, 2 * D + mc * 128:2 * D + (mc + 1) * 128],
                                             ones_r, start=True, stop=False)
                            for j in range(KP):
                                nc.tensor.matmul(ps[:], wk8t[j][:, :, mc * 128:(mc + 1) * 128],
                                                 h18[j][:, :, cg * 512:(cg + 1) * 512],
                                                 start=False, stop=(j == KP - 1), perf_mode=DR)
                            nc.scalar.copy(k8t[mc][:, cglob * 512:(cglob + 1) * 512], ps[:])
                    for tl in range(8):
                        tt = half * 8 + tl
                        for c in range(2):
                            ps = pmm.tile([128, 512], f32, tag="pmm")
                            nc.tensor.matmul(ps[:], ones_r[0:1, 0:128],
                                             rqkv[0:1, c * 512:(c + 1) * 512],
                                             start=True, stop=False)
                            for j in range(KP):
                                nc.tensor.matmul(ps[:], h18[j][:, :, tl * 128:(tl + 1) * 128],
                                                 wv8t[j][:, :, c * 512:(c + 1) * 512],
                                                 start=False, stop=(j == KP - 1), perf_mode=DR)
                            nc.scalar.copy(v_pair[tt // 2][:, tt % 2, c * 8:(c + 1) * 8, 0:64], ps[:])

            # ============ Scope 2: attention
            PAIRS = {0: [0, 1, 4, 5], 1: [0, 1, 2, 3, 4, 5, 6, 7]}
            GATED = {0: (4, 5), 1: (6, 7)}
            WAVES = {3: (0, 2), 7: (2, 4), 11: (4, 6), 13: (6, 7)}
            op_stack = contextlib.ExitStack()
            wop = op_stack.enter_context(tc.tile_pool(name="wo", bufs=1))
            t1p = op_stack.enter_context(tc.tile_pool(name="t1", bufs=1))
            pm2 = op_stack.enter_context(tc.tile_pool(name="pm2", bufs=1, space="PSUM"))
            bo_r = wop.tile([1, D], f32r, tag="bor")
            nc.sync.dma_start(out=bo_r[:], in_=rows[0:1, R_BO:R_BO + D])
            id_t = wop.tile([128, 128], f32r, tag="idt")
            nc.sync.dma_start(out=id_t[:], in_=cstr[:, 2048:2176])
            mask_t = []
            for j in range(4):
                m = wop.tile([128, 512], f32r, tag=f"mask{j}")
                nc.sync.dma_start(out=m[:], in_=cstr[:, j * 512:(j + 1) * 512])
                mask_t.append(m)
            wo8t = []
            for j in range(KT):
                wt = wop.tile([128, D], bf16, tag=f"wo{j}", name=f"wo{j}")
                nc.sync.dma_start(out=wt[:], in_=wo16[j])
                wo8t.append(wt)
            t1t = [t1p.tile([128, TQ], f32, tag=f"t1{i}", name=f"t1{i}") for i in range(KT)]
            at_stack = contextlib.ExitStack()
            atp = at_stack.enter_context(tc.tile_pool(name="at8", bufs=1))
            with contextlib.ExitStack() as s2:
                etp = s2.enter_context(tc.tile_pool(name="et", bufs=6))
                stp = s2.enter_context(tc.tile_pool(name="st2", bufs=2))
                pss = s2.enter_context(tc.tile_pool(name="pss", bufs=2, space="PSUM"))
                pav = s2.enter_context(tc.tile_pool(name="pav", bufs=2, space="PSUM"))
                attnb = [atp.tile([128, TQ], bf16, tag=f"at{i}", name=f"at{i}") for i in range(KT)]

                def outproj_partial(jlo, jhi, emit, pool, bufname=None):
                    for mc in range(KT):
                        ps = pool.tile([128, TQ], f32, tag="pm2",
                                       name=(f"{bufname}{mc}" if bufname else None))
                        if jlo == 0:
                            for c in range(2):
                                nc.tensor.matmul(ps[:, c * 512:(c + 1) * 512],
                                                 bo_r[0:1, mc * 128:(mc + 1) * 128],
                                                 ones_r, start=True, stop=False)
                        for j in range(jlo, jhi):
                            for c in range(2):
                                nc.tensor.matmul(ps[:, c * 512:(c + 1) * 512],
                                                 wo8t[j][:, mc * 128:(mc + 1) * 128],
                                                 attnb[j][:, c * 512:(c + 1) * 512],
                                                 start=(j == jlo and jlo != 0), stop=(j == jhi - 1))
                        emit(mc, ps)

                for h in range(H):
                    mcK = h // 2
                    off = (h % 2) * 64
                    for qc in range(2):
                        ets = {}
                        for p in PAIRS[qc]:
                            ps2 = pss.tile([128, 1024], f32, tag="pss")
                            for i in range(2):
                                kt = 2 * p + i
                                bnd = 4 * qc <= kt < 4 * (qc + 1)
                                dst = ps2[:, i * 512:(i + 1) * 512]
                                if bnd:
                                    nc.tensor.matmul(dst, id_t[:], mask_t[kt - 4 * qc][:], start=True, stop=False)
                                nc.tensor.matmul(dst, k8t[mcK][off:off + 64, kt * 128:(kt + 1) * 128],
                                                 q8t[mcK][off:off + 64, qc * 512:(qc + 1) * 512],
                                                 start=(not bnd), stop=True)
                            et = etp.tile([128, 2, 512], fp8, tag="et")
                            bias = pb_t[:, qc:qc + 1] if p in GATED[qc] else 0.0
                            nc.scalar.activation(et[:], ps2[:], Act.Exp, bias=bias, scale=0.125 / 1024.0)
                            ets[p] = et
                        ps_av = pav.tile([65, 512], f32, tag="pav")
                        vis = PAIRS[qc]
                        for idx, p in enumerate(vis):
                            nc.tensor.matmul(ps_av[:], v_pair[p][:, :, h, 0:65], ets[p][:],
                                             start=(idx == 0), stop=(idx == len(vis) - 1), perf_mode=DR)
                        rec = stp.tile([1, 512], f32, tag="rec")
                        nc.vector.reciprocal(rec[:], ps_av[64:65, :])
                        bc_sb = stp.tile([64, 512], f32, tag="bcsb")
                        nc.gpsimd.partition_broadcast(bc_sb[:], rec[:], channels=64)
                        nc.vector.tensor_tensor(attnb[mcK][off:off + 64, qc * 512:(qc + 1) * 512],
                                                ps_av[0:64, :], bc_sb[:], Alu.mult)
                    if h in WAVES:
                        jlo, jhi = WAVES[h]
                        if h == 3:
                            outproj_partial(jlo, jhi, lambda mc, ps: nc.vector.tensor_tensor(
                                t1t[mc][:], ps[:], xo[mc][:].bitcast(f32), Alu.add), pm2)
                        else:
                            outproj_partial(jlo, jhi, lambda mc, ps: nc.vector.tensor_tensor(
                                t1t[mc][:], ps[:], t1t[mc][:], Alu.add), pm2)

                # ============ Scope 2b: final out-proj wave (j=7) + combine -> x1
                # (x1 reuses the xo tiles in place: x1 = x + attn_out + bo)
                x1 = xo

                def emit_combine(mc, ps):
                    with nc.allow_low_precision(reason="x1 residual kept in f32r for LN2 stats matmuls"):
                        nc.vector.tensor_tensor(x1[mc][:], ps[:], t1t[mc][:], Alu.add)
                outproj_partial(KT - 1, KT, emit_combine, pm2)
            at_stack.close()
            op_stack.close()
            qkv_stack.close()

            # ============ Scope 3a: LN2 -> h2 (bf16)
            h2_stack = contextlib.ExitStack()
            h2p = h2_stack.enter_context(tc.tile_pool(name="h2", bufs=1))
            h2b = [h2p.tile([128, TQ], bf16, tag=f"h2{j}", name=f"h2t{j}") for j in range(KT)]
            with contextlib.ExitStack() as s3:
                P = ln_pools(s3)
                layernorm(P, lambda k: x1[k], lambda k: h2b[k][:])

            # ============ Scope 3b: MLP (bf16)
            with contextlib.ExitStack() as s3b:
                wfp = s3b.enter_context(tc.tile_pool(name="wf", bufs=4))
                mtp = s3b.enter_context(tc.tile_pool(name="mt", bufs=1))
                bfp = s3b.enter_context(tc.tile_pool(name="bf", bufs=1))
                evp = s3b.enter_context(tc.tile_pool(name="ev", bufs=3))
                pm3 = s3b.enter_context(tc.tile_pool(name="pm3", bufs=4, space="PSUM"))
                bfc_sb = bfp.tile([128, 32], f32, tag="bfs")
                nc.sync.dma_start(out=bfc_sb[:], in_=cstf[:, 2:34])
                bpr_r = bfp.tile([1, D], f32r, tag="bprr")
                nc.sync.dma_start(out=bpr_r[:], in_=rows[0:1, R_BPR:R_BPR + D])
                wpp2 = s3b.enter_context(tc.tile_pool(name="wpB", bufs=1))
                wprt = []
                for j in range(32):
                    wt = wpp2.tile([128, D], bf16, tag=f"wp{j}", name=f"wpt{j}")
                    nc.sync.dma_start(out=wt[:], in_=wpr16[j])
                    wprt.append(wt)
                mtb = [mtp.tile([128, TQ], bf16, tag=f"mt{j}", name=f"mtt{j}") for j in range(32)]
                for hc in range(32):
                    wf_t = wfp.tile([128, KT * 128], bf16, tag="wft")
                    nc.sync.dma_start(out=wf_t[:], in_=wfc16[hc])
                    ps = pm3.tile([128, TQ], f32, tag="pm3")
                    for j in range(KT):
                        for c in range(2):
                            nc.tensor.matmul(ps[:, c * 512:(c + 1) * 512],
                                             wf_t[:, j * 128:(j + 1) * 128],
                                             h2b[j][:, c * 512:(c + 1) * 512],
                                             start=(j == 0), stop=(j == KT - 1))
                    nc.scalar.activation(mtb[hc][:], ps[:], Act.Gelu, bias=bfc_sb[:, hc:hc + 1])
                for mc in range(KT):
                    ps = pm3.tile([128, TQ], f32, tag="pm3")
                    for c in range(2):
                        nc.tensor.matmul(ps[:, c * 512:(c + 1) * 512],
                                         bpr_r[0:1, mc * 128:(mc + 1) * 128],
                                         ones_r, start=True, stop=False)
                    for j in range(32):
                        for c in range(2):
                            nc.tensor.matmul(ps[:, c * 512:(c + 1) * 512],
                                             wprt[j][:, mc * 128:(mc + 1) * 128],
                                             mtb[j][:, c * 512:(c + 1) * 512],
                                             start=False, stop=(j == 31))
                    o = evp.tile([128, TQ], f32, tag="o")
                    nc.vector.tensor_tensor(o[:], ps[:], x1[mc][:].bitcast(f32), Alu.add)
                    nc.sync.dma_start(out=out_T[mc * 128:(mc + 1) * 128, :], in_=o[:])
            h2_stack.close()
            xo_stack.close()

    nc.compile()
    _CACHE["nc"] = nc
    return nc


# block assignment: a=0 -> query blocks (0, 3), others (1, 2), pb0=NEG, pb1=0
#                   a=1 -> query blocks (1, 2), others (0, 3), pb0=0,   pb1=NEG
BLOCKS = {0: ((0, 3), (1, 2)), 1: ((1, 2), (0, 3))}


def make_in_maps(x, ln1_g, ln1_b, wq, wk, wv, wo, bo, ln2_g, ln2_b, w_fc, b_fc, w_pr, b_pr):
    import ml_dtypes
    f8 = np.dtype(ml_dtypes.float8_e4m3)
    b16 = np.dtype(ml_dtypes.bfloat16)
    x = np.asarray(x, np.float32)

    def q8(a, npair):
        a = np.clip(np.asarray(a, np.float32), -240.0, 240.0).astype(f8)
        R, C = a.shape
        assert R == npair * 256
        return np.ascontiguousarray(a.reshape(npair, 2, 128, C).transpose(0, 2, 1, 3))

    def cols(b, n):
        return np.ascontiguousarray(np.asarray(b, np.float32).reshape(n, 128).T)

    mk = np.zeros((4, 128, 512), np.float32)
    for j in range(4):
        kp = np.arange(128)[:, None] + j * 128
        qf = np.arange(512)[None, :]
        mk[j] = np.where(kp <= qf, 0.0, NEG)

    g1v = np.asarray(ln1_g, np.float32)
    b1v = np.asarray(ln1_b, np.float32)
    g2v = np.asarray(ln2_g, np.float32)
    b2v = np.asarray(ln2_b, np.float32)
    wq2 = np.transpose(np.asarray(wq, np.float32), (1, 0, 2)).reshape(D, D)
    wk2 = np.transpose(np.asarray(wk, np.float32), (1, 0, 2)).reshape(D, D)
    wv2 = np.transpose(np.asarray(wv, np.float32), (1, 0, 2)).reshape(D, D)
    wfc2 = np.asarray(w_fc, np.float32)

    cstr = np.zeros((128, 2177), np.float32)
    cstr[:, 0:2048] = mk.transpose(1, 0, 2).reshape(128, 2048)
    cstr[:, 2048:2176] = np.eye(128, dtype=np.float32)
    cstr[:, 2176:2177] = 1.0
    rows = np.zeros((1, R_W), np.float32)
    rows[0, R_ONES:R_ONES + 512] = 1.0
    rows[0, R_BV:R_BV + D] = 32.0 * (b1v @ wv2)
    rows[0, R_BQ:R_BQ + D] = 32.0 * (b1v @ wq2)
    rows[0, R_BK:R_BK + D] = 32.0 * (b1v @ wk2)
    rows[0, R_BO:R_BO + D] = np.asarray(bo, np.float32)
    rows[0, R_BPR:R_BPR + D] = np.asarray(b_pr, np.float32)
    cstf = np.zeros((128, 34), np.float32)
    cstf[:, 2:34] = cols(np.asarray(b_fc, np.float32) + b2v @ wfc2, 32)
    wqkv8 = np.concatenate([
        q8(32.0 * g1v[:, None] * wq2, KP),
        q8(32.0 * g1v[:, None] * wk2, KP),
        q8(32.0 * g1v[:, None] * wv2, KP),
    ], 0)
    wb16 = np.concatenate([
        np.asarray(wo, np.float32).reshape(KT, 128, D).astype(b16),
        np.ascontiguousarray(
            (g2v[:, None] * wfc2).reshape(KT, 128, 32, 128).transpose(2, 1, 0, 3)
        ).reshape(32, 128, KT * 128).astype(b16),
        np.asarray(w_pr, np.float32).reshape(32, 128, D).astype(b16),
    ], 0)
    shared = {"cstr": cstr, "rows": rows, "wqkv8": wqkv8, "wb16": wb16}
    in_maps = []
    for b in range(4):
        for a in range(2):
            (g0, g1), (o0, o1) = BLOCKS[a]
            blk = lambda g: x[b, g * 512:(g + 1) * 512]
            m = dict(shared)
            m["xkv"] = np.ascontiguousarray(
                np.concatenate([blk(g0), blk(g1), blk(o0), blk(o1)], 0).T)
            cf = cstf.copy()
            cf[:, 0] = NEG if a == 0 else 0.0
            cf[:, 1] = 0.0 if a == 0 else NEG
            m["cstf"] = cf
            in_maps.append(m)
    return in_maps


def kernel(x, ln1_g, ln1_b, wq, wk, wv, wo, bo, ln2_g, ln2_b, w_fc, b_fc, w_pr, b_pr):
    from concourse.bass_utils import run_bass_kernel_spmd

    nc = _build()
    in_maps = make_in_maps(x, ln1_g, ln1_b, wq, wk, wv, wo, bo, ln2_g, ln2_b,
                           w_fc, b_fc, w_pr, b_pr)
    res = run_bass_kernel_spmd(nc, in_maps, list(range(8)))
    out = np.empty((4, 2048, D), np.float32)
    for b in range(4):
        for a in range(2):
            (g0, g1), _ = BLOCKS[a]
            oT = res.results[2 * b + a]["out_T"]
            out[b, g0 * 512:(g0 + 1) * 512, :] = oT[:, 0:512].T
            out[b, g1 * 512:(g1 + 1) * 512, :] = oT[:, 512:1024].T
    return out
